# revision 1
# baseline (speedup 1.0000x reference)
"""HGRN2Block kernel for 8 TRN2 NeuronCores.

Live path of the reference (the recurrence is dead code):
    x_proj = x @ W_proj + b_proj            # [B,L,3D]
    gate, _, ogate = split(x_proj, 3)       # middle third is DEAD
    out = (gate) * sigmoid(ogate)           # [B,L,D]
    out = out @ W_out + b_out               # [B,L,D]

Strategy:
  - Data-parallel over B*L rows: 16384 rows -> 2048 rows/core, no collectives.
  - Feature-major layout on device: host transposes x shard -> xT [D, rows]
    (free), so every matmul contracts over the SBUF partition dim without any
    on-device transpose. Output comes back as yT [D, rows]; host transposes.
  - bf16 matmuls (PSUM accumulates fp32): 4x the fp32 TensorE throughput.
  - Only the live 2/3 of W_proj is computed (cols [0:D] and [2D:3D]).
"""

import os

import numpy as np
import ml_dtypes

try:
    import concourse.bass as bass
except ImportError:
    import sys

    sys.path.insert(0, "/opt/trn_rl_repo")
    import concourse.bass as bass

import concourse.mybir as mybir
from concourse import bacc
from concourse.tile import TileContext
from concourse.bass_utils import run_bass_kernel_spmd

BF16 = ml_dtypes.bfloat16

B, L, D = 4, 4096, 1024
NCORES = 8
ROWS = B * L            # 16384
RPC = ROWS // NCORES    # 2048 rows per core
RB = 512                # moving free-dim per matmul (= one fp32 PSUM bank)
NRB = RPC // RB         # 4 row blocks per core
P = 128                 # SBUF partitions
KT = D // P             # 8 contraction tiles

_NC = None
LAST_RESULT = None      # BassKernelResults of the most recent run (for test.py)


def _build():
    nc = bacc.Bacc(trn_type="TRN2")
    f32 = mybir.dt.float32
    bf16 = mybir.dt.bfloat16

    xT = nc.dram_tensor("xT", [D, RPC], bf16, kind="ExternalInput")
    wg = nc.dram_tensor("wg", [D, D], bf16, kind="ExternalInput")
    wo = nc.dram_tensor("wo", [D, D], bf16, kind="ExternalInput")
    wout = nc.dram_tensor("wout", [D, D], bf16, kind="ExternalInput")
    bg = nc.dram_tensor("bg", [D], f32, kind="ExternalInput")
    bo = nc.dram_tensor("bo", [D], f32, kind="ExternalInput")
    bout = nc.dram_tensor("bout", [D], f32, kind="ExternalInput")
    yT = nc.dram_tensor("yT", [D, RPC], f32, kind="ExternalOutput")

    with TileContext(nc) as tc:
        with (
            tc.tile_pool(name="const", bufs=1) as cpool,
            tc.tile_pool(name="work", bufs=2) as wpool,
            tc.tile_pool(name="outp", bufs=4) as opool,
            tc.tile_pool(name="ps", bufs=2, space="PSUM") as pspool,
        ):
            # Biases: [D] -> SBUF [128, KT]; column k holds features k*128..
            bgS = cpool.tile([P, KT], f32, tag="bg", name="bgS")
            boS = cpool.tile([P, KT], f32, tag="bo", name="boS")
            boutS = cpool.tile([P, KT], f32, tag="bout", name="boutS")
            nc.gpsimd.dma_start(out=bgS, in_=bg[:].rearrange("(k p) -> p k", p=P))
            nc.gpsimd.dma_start(out=boS, in_=bo[:].rearrange("(k p) -> p k", p=P))
            nc.gpsimd.dma_start(out=boutS, in_=bout[:].rearrange("(k p) -> p k", p=P))

            # Warm-up: HAM starts the PE clock-gated at 1.2 GHz and only
            # ungates after ~3.4us of sustained activity. Spin matmuls on a
            # zeroed tile (no DMA deps) so the PE is warm before real work.
            wz = cpool.tile([P, RB], bf16, tag="wz", name="wz")
            nc.vector.memset(wz, 0.0)
            spin = pspool.tile([P, RB], f32, tag="spin", name="spin", bufs=1)
            for _ in range(28):
                nc.tensor.matmul(spin, lhsT=wz[:, :P], rhs=wz, start=True, stop=True)

            # x (feature-major) and all three weight matrices, resident in
            # SBUF. DMA order = need order: wg + rb0 columns of x first (the
            # first PSUM group needs all 8 k-tiles of wg), wout last.
            xS = [cpool.tile([P, RPC], bf16, tag=f"x{k}", name=f"xS{k}") for k in range(KT)]
            wgS = [cpool.tile([P, D], bf16, tag=f"wg{k}", name=f"wgS{k}") for k in range(KT)]
            woS = [cpool.tile([P, D], bf16, tag=f"wo{k}", name=f"woS{k}") for k in range(KT)]
            woutS = [cpool.tile([P, D], bf16, tag=f"wu{k}", name=f"wuS{k}") for k in range(KT)]
            # Queue roles: gpsimd carries weights (need order wo, wg, wout),
            # sync carries activations then shares with outputs, scalar's
            # queue stays empty so sigmoids never queue behind DMA triggers.
            for k in range(KT):
                r = slice(k * P, (k + 1) * P)
                nc.gpsimd.dma_start(out=woS[k], in_=wo[r, :])
                nc.sync.dma_start(out=xS[k][:, 0:RB], in_=xT[r, 0:RB])
            for k in range(KT):
                nc.gpsimd.dma_start(out=wgS[k], in_=wg[slice(k * P, (k + 1) * P), :])
            for rb in range(1, NRB):
                c = slice(rb * RB, (rb + 1) * RB)
                for k in range(KT):
                    nc.sync.dma_start(out=xS[k][:, c], in_=xT[slice(k * P, (k + 1) * P), c])
            for k in range(KT):
                nc.gpsimd.dma_start(out=woutS[k], in_=wout[slice(k * P, (k + 1) * P), :])

            for rb in range(NRB):
                cols = slice(rb * RB, (rb + 1) * RB)
                # ---- layer 1: hT/oT tiles -> gT = (hT+bg) * sigmoid(oT+bo)
                gS = [wpool.tile([P, RB], bf16, tag=f"g{m}", name=f"gS{rb}_{m}") for m in range(KT)]
                for m in range(KT):
                    msl = slice(m * P, (m + 1) * P)
                    ph = pspool.tile([P, RB], f32, tag="ph", name=f"ph{rb}_{m}")
                    po = pspool.tile([P, RB], f32, tag="po", name=f"po{rb}_{m}")
                    # o-group first: its sigmoid (ScalarE) overlaps the h-group
                    for k in range(KT):
                        nc.tensor.matmul(
                            po, lhsT=woS[k][:, msl], rhs=xS[k][:, cols],
                            start=(k == 0), stop=(k == KT - 1),
                        )
                    for k in range(KT):
                        nc.tensor.matmul(
                            ph, lhsT=wgS[k][:, msl], rhs=xS[k][:, cols],
                            start=(k == 0), stop=(k == KT - 1),
                        )
                    sig = opool.tile([P, RB], bf16, tag="sig", name=f"sig{rb}_{m}")
                    nc.scalar.activation(
                        out=sig, in_=po,
                        func=mybir.ActivationFunctionType.Sigmoid,
                        bias=boS[:, m : m + 1], scale=1.0,
                    )
                    nc.vector.scalar_tensor_tensor(
                        out=gS[m], in0=ph, scalar=bgS[:, m : m + 1], in1=sig,
                        op0=mybir.AluOpType.add, op1=mybir.AluOpType.mult,
                    )
                # ---- layer 2: yT = gT.T-contract @ W_out (+ b_out)
                for n in range(KT):
                    nsl = slice(n * P, (n + 1) * P)
                    py = pspool.tile([P, RB], f32, tag="py", name=f"py{rb}_{n}")
                    for m in range(KT):
                        nc.tensor.matmul(
                            py, lhsT=woutS[m][:, nsl], rhs=gS[m],
                            start=(m == 0), stop=(m == KT - 1),
                        )
                    yo = opool.tile([P, RB], f32, tag="yo", name=f"yo{rb}_{n}")
                    # bias-add on DVE, keeping ScalarE free for sigmoids
                    nc.vector.tensor_scalar_add(yo, py, boutS[:, n : n + 1])
                    nc.sync.dma_start(out=yT[nsl, cols], in_=yo)
    nc.finalize()
    return nc


def kernel(x, W_proj, b_proj, W_out, b_out, layer_idx=0, num_layers=12):
    global _NC, LAST_RESULT
    x = np.asarray(x, dtype=np.float32)
    W_proj = np.asarray(W_proj, dtype=np.float32)
    b_proj = np.asarray(b_proj, dtype=np.float32)
    W_out = np.asarray(W_out, dtype=np.float32)
    b_out = np.asarray(b_out, dtype=np.float32)

    wg = W_proj[:, :D].astype(BF16)
    wo = W_proj[:, 2 * D : 3 * D].astype(BF16)
    wu = W_out.astype(BF16)
    bg = np.ascontiguousarray(b_proj[:D])
    bo = np.ascontiguousarray(b_proj[2 * D : 3 * D])
    bu = np.ascontiguousarray(b_out)

    xf = x.reshape(ROWS, D)
    in_maps = []
    for c in range(NCORES):
        xs = xf[c * RPC : (c + 1) * RPC, :]
        xT = xs.T.astype(BF16)  # astype copies -> C-contiguous [D, RPC]
        in_maps.append(
            {"xT": xT, "wg": wg, "wo": wo, "wout": wu,
             "bg": bg, "bo": bo, "bout": bu}
        )

    if _NC is None:
        _NC = _build()

    trace = os.environ.get("HGRN_TRACE", "0") == "1"
    LAST_RESULT = run_bass_kernel_spmd(
        _NC, in_maps, core_ids=list(range(NCORES)), trace=trace,
        tmpdir=os.environ.get("HGRN_TMPDIR"),
    )
    y = np.empty((ROWS, D), dtype=np.float32)
    for c in range(NCORES):
        y[c * RPC : (c + 1) * RPC, :] = np.asarray(
            LAST_RESULT.results[c]["yT"], dtype=np.float32
        ).T
    return y.reshape(B, L, D)



# revision 2
# speedup vs baseline: 1.1633x; 1.1633x over previous
"""HGRN2Block kernel for 8 TRN2 NeuronCores.

Live path of the reference (the recurrence is dead code):
    x_proj = x @ W_proj + b_proj            # [B,L,3D]
    gate, _, ogate = split(x_proj, 3)       # middle third is DEAD
    out = gate * sigmoid(ogate)             # [B,L,D]
    out = out @ W_out + b_out               # [B,L,D]

Strategy:
  - Data-parallel over B*L rows: 16384 rows -> 2048 rows/core, no collectives.
  - Feature-major on device; host packs every tensor so each DMA is one fully
    contiguous block (1 MB transfers run at ~341 GB/s vs ~14 GB/s for the
    row-strided patterns they replace).
  - Weights are packed m-major ([m][p][k][c]) so the first matmul group only
    needs a 128 KB tile, not the whole 2 MB matrix -> compute starts ~8 us in.
  - The output-gate projection runs in fp8 e4m3 with DoubleRow (2 k-slices per
    matmul): its quantization error is damped by sigmoid' (~0.21 RMS), giving
    rel_err ~1.5e-2 (host-simulated) vs the 2e-2 budget. The gate projection
    and output projection stay bf16 (fp8 there would blow the budget).
  - o-proj weights pre-scaled x16 into e4m3's normal range; the sigmoid
    activation descales via its scale operand: sigmoid(psum/16 + bias).
  - Queues: sync (HWDGE) carries x then y-out; gpsimd (SWDGE) carries weights;
    scalar (HWDGE) carries the three tiny bias tiles then only sigmoids.
  - Per row-block: all 8 fp8 o-groups first (need only the small fp8 tiles),
    then 8 bf16 h-groups, then 8 bf16 layer-2 groups. Output DMA'd as bf16.
"""

import os

import numpy as np
import ml_dtypes

try:
    import concourse.bass as bass
except ImportError:
    import sys

    sys.path.insert(0, "/opt/trn_rl_repo")
    import concourse.bass as bass

import concourse.mybir as mybir
from concourse import bacc
from concourse.tile import TileContext
from concourse.bass_utils import run_bass_kernel_spmd

BF16 = ml_dtypes.bfloat16
F8E4 = ml_dtypes.float8_e4m3  # TRN fp8_e4m3 variant (max +-240)

B, L, D = 4, 4096, 1024
NCORES = 8
ROWS = B * L            # 16384
RPC = ROWS // NCORES    # 2048 rows per core
RB = 512                # moving free-dim per matmul (= one fp32 PSUM bank)
NRB = RPC // RB         # 4 row blocks per core
P = 128                 # SBUF partitions
KT = D // P             # 8 contraction tiles
WSCALE = 16.0           # o-proj weight pre-scale (descaled in the sigmoid)

_NC = None
LAST_RESULT = None      # BassKernelResults of the most recent run (for test.py)


def _build():
    nc = bacc.Bacc(trn_type="TRN2")
    f32 = mybir.dt.float32
    bf16 = mybir.dt.bfloat16
    f8 = mybir.dt.float8e4
    DR = mybir.MatmulPerfMode.DoubleRow

    x8 = nc.dram_tensor("x8", [NRB, P, KT, RB], f8, kind="ExternalInput")
    xb = nc.dram_tensor("xb", [NRB, P, KT, RB], bf16, kind="ExternalInput")
    wo8 = nc.dram_tensor("wo8", [KT, P, KT, P], f8, kind="ExternalInput")
    wg = nc.dram_tensor("wg", [KT, P, KT, P], bf16, kind="ExternalInput")
    wu = nc.dram_tensor("wu", [KT, P, KT, P], bf16, kind="ExternalInput")
    bg = nc.dram_tensor("bg", [P, KT], f32, kind="ExternalInput")
    bo = nc.dram_tensor("bo", [P, KT], f32, kind="ExternalInput")
    bu = nc.dram_tensor("bu", [P, KT], f32, kind="ExternalInput")
    y = nc.dram_tensor("y", [NRB, KT, P, RB], bf16, kind="ExternalOutput")

    with TileContext(nc) as tc:
        with (
            tc.tile_pool(name="const", bufs=1) as cpool,
            tc.tile_pool(name="work", bufs=2) as wpool,
            tc.tile_pool(name="outp", bufs=2) as opool,
            tc.tile_pool(name="ps", bufs=2, space="PSUM") as pspool,
        ):
            # Biases land as contiguous [128, 8] tiles (host pre-transposed);
            # column m holds features m*128..m*128+127.
            bgS = cpool.tile([P, KT], f32, tag="bg", name="bgS")
            boS = cpool.tile([P, KT], f32, tag="bo", name="boS")
            buS = cpool.tile([P, KT], f32, tag="bu", name="buS")
            nc.scalar.dma_start(out=bgS, in_=bg[:, :])
            nc.scalar.dma_start(out=boS, in_=bo[:, :])
            nc.scalar.dma_start(out=buS, in_=bu[:, :])

            # Warm-up: HAM starts the PE clock-gated at 1.2 GHz and ungates
            # after ~3.4us of sustained activity. A few spins on a zeroed tile
            # (no DMA deps) start the warm-up clock during the DMA prologue.
            wz = cpool.tile([P, RB], bf16, tag="wz", name="wz")
            nc.vector.memset(wz, 0.0)
            spin = pspool.tile([P, RB], f32, tag="spin", name="spin", bufs=1)
            for _ in range(4):
                nc.tensor.matmul(spin, lhsT=wz[:, :P], rhs=wz, start=True, stop=True)

            # All inputs SBUF-resident. DMA order = need order.
            x8S = [cpool.tile([P, KT, RB], f8, tag=f"x8_{r}", name=f"x8S{r}")
                   for r in range(NRB)]
            xbS = [cpool.tile([P, KT, RB], bf16, tag=f"xb_{r}", name=f"xbS{r}")
                   for r in range(NRB)]
            wo8S = [cpool.tile([P, KT, P], f8, tag=f"wo8_{m}", name=f"wo8S{m}")
                    for m in range(KT)]
            wgS = [cpool.tile([P, KT, P], bf16, tag=f"wg_{m}", name=f"wgS{m}")
                   for m in range(KT)]
            wuS = [cpool.tile([P, KT, P], bf16, tag=f"wu_{n}", name=f"wuS{n}")
                   for n in range(KT)]

            # sync HWDGE: x tiles; the first fp8/bf16 tiles split in halves so
            # the first o-group can start after ~256 KB instead of 512 KB.
            H = KT // 2
            nc.sync.dma_start(out=x8S[0][:, 0:H, :], in_=x8[0, :, 0:H, :])
            nc.sync.dma_start(out=x8S[0][:, H:KT, :], in_=x8[0, :, H:KT, :])
            nc.sync.dma_start(out=xbS[0][:, 0:H, :], in_=xb[0, :, 0:H, :])
            nc.sync.dma_start(out=xbS[0][:, H:KT, :], in_=xb[0, :, H:KT, :])
            for r in range(1, NRB):
                nc.sync.dma_start(out=x8S[r], in_=x8[r, :, :, :])
                nc.sync.dma_start(out=xbS[r], in_=xb[r, :, :, :])

            # gpsimd SWDGE: weights. All fp8 o-proj tiles first (the whole
            # o-phase of rb0 needs only these 1 MB), then gate, then output.
            for m in range(KT):
                nc.gpsimd.dma_start(out=wo8S[m], in_=wo8[m, :, :, :])
            for m in range(KT):
                nc.gpsimd.dma_start(out=wgS[m], in_=wg[m, :, :, :])
            for n in range(KT):
                nc.gpsimd.dma_start(out=wuS[n], in_=wu[n, :, :, :])

            for rb in range(NRB):
                # ---- o-phase: fp8 DoubleRow, 4 matmuls per group ----
                sigs = []
                for m in range(KT):
                    po = pspool.tile([P, RB], f32, tag="po", name=f"po{rb}_{m}")
                    for j in range(KT // 2):
                        nc.tensor.matmul(
                            po,
                            lhsT=wo8S[m][:, 2 * j:2 * j + 2, :],
                            rhs=x8S[rb][:, 2 * j:2 * j + 2, :],
                            start=(j == 0), stop=(j == KT // 2 - 1),
                            perf_mode=DR,
                        )
                    sig = opool.tile([P, RB], bf16, tag=f"sig{m}",
                                     name=f"sig{rb}_{m}")
                    nc.scalar.activation(
                        out=sig, in_=po,
                        func=mybir.ActivationFunctionType.Sigmoid,
                        bias=boS[:, m:m + 1], scale=1.0 / WSCALE,
                    )
                    sigs.append(sig)
                # ---- h-phase: bf16 gate proj; g = (h + bg) * sig ----
                gS = []
                for m in range(KT):
                    ph = pspool.tile([P, RB], f32, tag="ph", name=f"ph{rb}_{m}")
                    for k in range(KT):
                        nc.tensor.matmul(
                            ph, lhsT=wgS[m][:, k:k + 1, :],
                            rhs=xbS[rb][:, k:k + 1, :],
                            start=(k == 0), stop=(k == KT - 1),
                        )
                    g = wpool.tile([P, RB], bf16, tag=f"g{m}", name=f"g{rb}_{m}")
                    nc.vector.scalar_tensor_tensor(
                        out=g, in0=ph, scalar=bgS[:, m:m + 1], in1=sigs[m],
                        op0=mybir.AluOpType.add, op1=mybir.AluOpType.mult,
                    )
                    gS.append(g)
                # ---- layer 2: y = g @ W_out (+ b_out), bf16 out ----
                for n in range(KT):
                    py = pspool.tile([P, RB], f32, tag="py", name=f"py{rb}_{n}")
                    for m in range(KT):
                        nc.tensor.matmul(
                            py, lhsT=wuS[n][:, m:m + 1, :], rhs=gS[m],
                            start=(m == 0), stop=(m == KT - 1),
                        )
                    yo = opool.tile([P, RB], bf16, tag="yo", name=f"yo{rb}_{n}",
                                    bufs=4)
                    nc.vector.tensor_scalar_add(yo, py, buS[:, n:n + 1])
                    nc.sync.dma_start(out=y[rb, n, :, :], in_=yo)
    nc.finalize()
    return nc


def kernel(x, W_proj, b_proj, W_out, b_out, layer_idx=0, num_layers=12):
    global _NC, LAST_RESULT
    x = np.asarray(x, dtype=np.float32)
    W_proj = np.asarray(W_proj, dtype=np.float32)
    b_proj = np.asarray(b_proj, dtype=np.float32)
    W_out = np.asarray(W_out, dtype=np.float32)
    b_out = np.asarray(b_out, dtype=np.float32)

    Wg = W_proj[:, :D]
    Wo = W_proj[:, 2 * D:3 * D]

    def pack_w(w):
        # [D, D] -> [a][p][b][c] with out[a, p, b, c] = w[b*128+p, a*128+c]
        return np.ascontiguousarray(w.reshape(KT, P, KT, P).transpose(2, 1, 0, 3))

    wgp = pack_w(Wg).astype(BF16)
    wo8p = pack_w(Wo * WSCALE).astype(F8E4)
    wup = pack_w(W_out).astype(BF16)
    bgp = np.ascontiguousarray(b_proj[:D].reshape(KT, P).T)
    bop = np.ascontiguousarray(b_proj[2 * D:3 * D].reshape(KT, P).T)
    bup = np.ascontiguousarray(b_out.reshape(KT, P).T)

    xf = x.reshape(ROWS, D)
    in_maps = []
    for c in range(NCORES):
        # [rb, p, k, cb] with xc[rb, p, k, cb] = x_core[rb*512+cb, k*128+p]
        xc = np.ascontiguousarray(
            xf[c * RPC:(c + 1) * RPC].reshape(NRB, RB, KT, P).transpose(0, 3, 2, 1)
        )
        in_maps.append({
            "x8": xc.astype(F8E4), "xb": xc.astype(BF16),
            "wo8": wo8p, "wg": wgp, "wu": wup,
            "bg": bgp, "bo": bop, "bu": bup,
        })

    if _NC is None:
        _NC = _build()

    trace = os.environ.get("HGRN_TRACE", "0") == "1"
    LAST_RESULT = run_bass_kernel_spmd(
        _NC, in_maps, core_ids=list(range(NCORES)), trace=trace,
        tmpdir=os.environ.get("HGRN_TMPDIR"),
    )
    yout = np.empty((ROWS, D), dtype=np.float32)
    for c in range(NCORES):
        yc = np.asarray(LAST_RESULT.results[c]["y"])  # [rb, n, p, cb] bf16
        yout[c * RPC:(c + 1) * RPC] = (
            yc.transpose(0, 3, 1, 2).reshape(RPC, D).astype(np.float32)
        )
    return yout.reshape(B, L, D)
